# revision 1
# baseline (speedup 1.0000x reference)
# Trainium2 Bass kernel for nn_FuzzyNeuralNework (moe_routing).
#
# Math (reference):
#   logits[b,r] = sum_d -(x[b,d]-cen[d,r])^2 / (2 sig[d,r]^2)
#   raw = exp(logits) * mask ;  frs = raw / (sum_r raw + 1e-10)
#   xn = batchnorm(x) (global batch stats, biased var)
#   out[b,c] = sum_r frs[b,r] * (xn @ W[r])[b,c] + sum_r frs[b,r]*bias[r,c]
#
# Kernel restructuring:
#   logits^T = A^T x2^T + Bc^T x^T + k,  A=-1/(2 sig^2), Bc=cen/sig^2,
#       k[r] = sum_d -cen^2/(2 sig^2)   (two PE matmuls in [r,b] layout)
#   denom via a K=R matmul with rule_masks as the stationary vector
#   frs^T = (raw * mask) * exp(-ln(denom))  (one fused DVE stt; the 1/denom
#       row is partition-replicated via a DRAM-bounce broadcast DMA)
#   gating folded into the GEMM:  out^T[c,b] = sum_r W[r]^T @ (xn^T * frs^T[r,:])
#       accumulated over rules in PSUM; the two b-halves run on different
#       PE column groups (tile_position) so their streams overlap.
#   frs row replicas for the gating multiply are produced by broadcast DMAs
#   (compute engines cannot replicate across partitions).
#
# Sharding: batch B=8192 split across 8 cores (1024 each); small tensors
# replicated; BN stats computed on every core from the full (replicated) x^T
# (ACT Square+accum for sum(x^2), GpSimd reduce for sum(x)).

import numpy as np

B, D, R, C = 8192, 128, 64, 64
NCORES = 8
BL = B // NCORES
BN_EPS = 1e-5

_CACHE = {}


def _build_bass():
    import concourse.bass as bass
    import concourse.tile as tile
    from concourse import bacc, mybir

    f32 = mybir.dt.float32
    bf16 = mybir.dt.bfloat16
    AF = mybir.ActivationFunctionType
    OP = mybir.AluOpType

    nc = bacc.Bacc(
        "TRN2", target_bir_lowering=False, debug=False, num_devices=NCORES
    )

    d_xtf = nc.dram_tensor("xt_full", [D, B], f32, kind="ExternalInput").ap()
    d_xtl = nc.dram_tensor("xt_loc", [D, BL], f32, kind="ExternalInput").ap()
    d_cen = nc.dram_tensor("centers_t", [D, R], f32, kind="ExternalInput").ap()
    d_sig = nc.dram_tensor("sigmas_t", [D, R], f32, kind="ExternalInput").ap()
    d_wst = nc.dram_tensor("wstack", [D, R * C], f32, kind="ExternalInput").ap()
    d_b2d = nc.dram_tensor("biases2d", [R, C], f32, kind="ExternalInput").ap()
    d_gam = nc.dram_tensor("gamma_c", [D, 1], f32, kind="ExternalInput").ap()
    d_bet = nc.dram_tensor("beta_c", [D, 1], f32, kind="ExternalInput").ap()
    d_msk = nc.dram_tensor("masks_c", [R, 1], f32, kind="ExternalInput").ap()
    d_out = nc.dram_tensor("outT", [C, BL], f32, kind="ExternalOutput").ap()

    with tile.TileContext(nc) as tc:
        with (
            tc.tile_pool(name="singles", bufs=1) as singles,
            tc.tile_pool(name="bigs", bufs=1) as bigs,
            tc.tile_pool(name="gpool", bufs=8) as gpool,
        ):
            ps_early_cm = tc.tile_pool(name="ps_early", bufs=1, space="PSUM")
            ps_small = ps_early_cm.__enter__()
            ps_logp = ps_small

            # ---- input DMAs (critical-path first, spread over engines) --
            sb_xtl = bigs.tile([D, BL], f32)
            nc.sync.dma_start(out=sb_xtl, in_=d_xtl)
            sb_cen = singles.tile([D, R], f32)
            sb_sig = singles.tile([D, R], f32)
            nc.scalar.dma_start(out=sb_cen, in_=d_cen)
            nc.scalar.dma_start(out=sb_sig, in_=d_sig)
            sb_gam = singles.tile([D, 1], f32)
            sb_bet = singles.tile([D, 1], f32)
            sb_msk = singles.tile([R, 1], f32)
            sb_b2d = singles.tile([R, C], f32)
            nc.gpsimd.dma_start(out=sb_gam, in_=d_gam)
            nc.gpsimd.dma_start(out=sb_bet, in_=d_bet)
            nc.scalar.dma_start(out=sb_msk, in_=d_msk)
            nc.gpsimd.dma_start(out=sb_b2d, in_=d_b2d)

            sb_xtf = bigs.tile([D, B], f32)
            dma_engs = [nc.sync, nc.scalar, nc.gpsimd]
            for h in range(4):
                sl = slice(h * (B // 4), (h + 1) * (B // 4))
                dma_engs[h % 2].dma_start(out=sb_xtf[:, sl], in_=d_xtf[:, sl])
            sb_wst = bigs.tile([D, R * C], f32)
            for h in range(4):
                sl = slice(h * (R * C // 4), (h + 1) * (R * C // 4))
                dma_engs[(h % 2)].dma_start(out=sb_wst[:, sl], in_=d_wst[:, sl])

            # ---- PE warmup (HAM) while DMAs stream in -------------------
            warm = singles.tile([D, 128], bf16)
            nc.gpsimd.memset(warm, 0.0)
            warm_ps = ps_small.tile([D, 128], f32)
            for _ in range(24):
                nc.tensor.matmul(warm_ps, warm, warm, start=True, stop=True)

            # ---- Gaussian-membership coefficient prep (tiny DVE ops) ----
            sigsq = singles.tile([D, R], f32)
            nc.vector.tensor_mul(sigsq, sb_sig, sb_sig)
            recs = singles.tile([D, R], f32)
            nc.vector.reciprocal(recs, sigsq)
            sbA = singles.tile([D, R], f32)
            nc.vector.tensor_scalar_mul(sbA, recs, -0.5)
            sbBc = singles.tile([D, R], f32)
            nc.vector.tensor_mul(sbBc, sb_cen, recs)
            csq = singles.tile([D, R], f32)
            nc.vector.tensor_mul(csq, sb_cen, sb_cen)
            cA = singles.tile([D, R], f32)
            nc.vector.tensor_mul(cA, csq, sbA)

            ones_d = singles.tile([D, 1], f32)
            nc.vector.memset(ones_d, 1.0)
            ps_k = ps_small.tile([R, 1], f32)
            nc.tensor.matmul(ps_k, cA, ones_d, start=True, stop=True)
            sb_k = singles.tile([R, 1], f32)
            nc.vector.tensor_copy(sb_k, ps_k)

            # ---- logits^T in PSUM [R, BL] (fp32 matmuls: exp-sensitive) --
            xsq_l = bigs.tile([D, BL], f32)
            nc.scalar.activation(xsq_l, sb_xtl, AF.Square)
            ps_log = ps_logp.tile([R, BL], f32)
            for h in range(2):
                sl = slice(h * 512, (h + 1) * 512)
                nc.tensor.matmul(
                    ps_log[:, sl], sbA, xsq_l[:, sl], start=True, stop=False
                )
                nc.tensor.matmul(
                    ps_log[:, sl], sbBc, sb_xtl[:, sl], start=False, stop=True
                )

            # raw = exp(logits + k)  (fp32; matches reference underflow
            # behaviour -- deliberately no max-subtraction)
            raw = bigs.tile([R, BL], f32)
            nc.scalar.activation(raw, ps_log, AF.Exp, bias=sb_k)

            # denom = sum_r mask_r * raw_r  (K=R matmul, masks as weights)
            ps_den = ps_small.tile([1, BL], f32)
            for h in range(2):
                sl = slice(h * 512, (h + 1) * 512)
                nc.tensor.matmul(
                    ps_den[:, sl], sb_msk, raw[:, sl], start=True, stop=True
                )
            eps_1 = singles.tile([1, 1], f32)
            nc.vector.memset(eps_1, 1e-10)
            lnd = singles.tile([1, BL], f32)
            nc.scalar.activation(lnd, ps_den, AF.Ln, bias=eps_1)
            # 1/denom = exp(-ln(denom)); broadcast to the 64 rule rows via a
            # DRAM-bounce DMA (compute engines cannot partition-broadcast).
            recip = singles.tile([1, BL], f32)
            nc.scalar.activation(recip, lnd, AF.Exp, scale=-1.0)
            dram_cm = tc.tile_pool(name="dram", bufs=1, space="DRAM")
            drams = dram_cm.__enter__()
            recip_dram = drams.tile([1, BL], f32)
            nc.sync.dma_start(out=recip_dram, in_=recip)
            recip_rep = bigs.tile([R, BL], f32)
            nc.sync.dma_start(
                out=recip_rep, in_=recip_dram[0:1, :].to_broadcast((R, BL))
            )
            # frs^T (bf16) = (raw * mask) * (1/denom)  in one fused DVE op
            frsm = bigs.tile([R, BL], bf16)
            nc.vector.scalar_tensor_tensor(
                out=frsm, in0=raw, scalar=sb_msk, in1=recip_rep,
                op0=OP.mult, op1=OP.mult,
            )
            frs_dram = drams.tile([R, BL], bf16)
            nc.sync.dma_start(out=frs_dram, in_=frsm)

            # ---- BN stats over the full batch (replicated) --------------
            # sum(x^2): two chunked ACT Square passes with accumulate
            # (scratch out), interleaved with the frs-critical ACT ops.
            sq_scratch = bigs.tile([D, B], bf16)
            sq_sums = singles.tile([D, 2], f32)
            for h in range(2):
                sl = slice(h * (B // 2), (h + 1) * (B // 2))
                nc.scalar.activation(
                    out=sq_scratch[:, sl], in_=sb_xtf[:, sl], func=AF.Square,
                    accum_out=sq_sums[:, h : h + 1],
                )
            # sum(x): chunked DVE reduces (fit in the idle pre-gating window)
            x_sums = singles.tile([D, 4], f32)
            for h in range(4):
                sl = slice(h * (B // 4), (h + 1) * (B // 4))
                nc.vector.tensor_reduce(
                    out=x_sums[:, h : h + 1], in_=sb_xtf[:, sl],
                    axis=mybir.AxisListType.X, op=OP.add,
                )
            x_sum = singles.tile([D, 1], f32)
            nc.vector.tensor_reduce(
                out=x_sum, in_=x_sums, axis=mybir.AxisListType.X, op=OP.add
            )
            sq_sum = singles.tile([D, 1], f32)
            nc.vector.tensor_reduce(
                out=sq_sum, in_=sq_sums, axis=mybir.AxisListType.X, op=OP.add
            )
            mean = singles.tile([D, 1], f32)
            nc.vector.tensor_scalar_mul(mean, x_sum, 1.0 / float(B))
            var = singles.tile([D, 1], f32)
            msq = singles.tile([D, 1], f32)
            nc.vector.tensor_mul(msq, mean, mean)
            nc.vector.tensor_scalar_mul(var, sq_sum, 1.0 / float(B))
            nc.vector.tensor_sub(var, var, msq)
            # rstd = exp(-0.5 * ln(var + eps)) : avoids the low-precision
            # Rsqrt table and shares the natural_log_exp ACT table set.
            eps_d = singles.tile([D, 1], f32)
            nc.vector.memset(eps_d, float(BN_EPS))
            lnv = singles.tile([D, 1], f32)
            nc.scalar.activation(lnv, var, AF.Ln, bias=eps_d)
            rstd = singles.tile([D, 1], f32)
            nc.scalar.activation(rstd, lnv, AF.Exp, scale=-0.5)
            a_sc = singles.tile([D, 1], f32)
            nc.vector.tensor_mul(a_sc, rstd, sb_gam)
            mu_a = singles.tile([D, 1], f32)
            nc.vector.tensor_mul(mu_a, mean, a_sc)
            c0 = singles.tile([D, 1], f32)
            nc.vector.tensor_sub(c0, sb_bet, mu_a)

            xn_bf = bigs.tile([D, BL], bf16)
            nc.vector.tensor_scalar(
                out=xn_bf, in0=sb_xtl, scalar1=a_sc, scalar2=c0,
                op0=OP.mult, op1=OP.add,
            )

            # ---- bf16 copies of the GEMM operands (GpSimd + DVE) --------
            wst_bf = bigs.tile([D, R * C], bf16)
            nc.gpsimd.tensor_copy(wst_bf, sb_wst)
            b2d_bf = singles.tile([R, C], bf16)
            nc.vector.tensor_copy(b2d_bf, sb_b2d)

            # ---- gated GEMM: out^T[c,b] accumulated over rules ----------
            # b-half 0 runs on PE column group 0 (psum partitions 0:64),
            # b-half 1 on column group 1 (psum partitions 64:128) so the two
            # matmul streams of each rule can overlap on the array.
            ps_early_cm.__exit__(None, None, None)
            ps_acc_cm = tc.tile_pool(name="ps_acc", bufs=1, space="PSUM")
            ps_accp = ps_acc_cm.__enter__()
            ps_out = ps_accp.tile([2 * C, BL], f32)
            sl0 = slice(0, 512)
            sl1 = slice(512, 1024)
            with tc.tile_pool(name="reps", bufs=8) as reps:
                for r in range(R):
                    rep = reps.tile([D, BL], bf16)
                    dma_engs[r % 3].dma_start(
                        out=rep,
                        in_=frs_dram[r : r + 1, :].to_broadcast((D, BL)),
                    )
                    g = gpool.tile([D, BL], bf16)
                    eng = nc.gpsimd if (r % 5 == 4) else nc.vector
                    eng.tensor_mul(g, xn_bf, rep)
                    wsl = wst_bf[:, r * C : (r + 1) * C]
                    nc.tensor.matmul(
                        ps_out[0:C, sl0], wsl, g[:, sl0],
                        start=(r == 0), stop=False, tile_position=(0, 0),
                    )
                    nc.tensor.matmul(
                        ps_out[C : 2 * C, sl1], wsl, g[:, sl1],
                        start=(r == 0), stop=False, tile_position=(0, 64),
                    )
            # bias term: out^T += biases2d^T @ frs^T  (closes both groups)
            nc.tensor.matmul(
                ps_out[0:C, sl0], b2d_bf, frsm[:, sl0],
                start=False, stop=True, tile_position=(0, 0),
            )
            nc.tensor.matmul(
                ps_out[C : 2 * C, sl1], b2d_bf, frsm[:, sl1],
                start=False, stop=True, tile_position=(0, 64),
            )

            # ---- evacuate + store --------------------------------------
            outf = bigs.tile([2 * C, BL], f32)
            nc.scalar.copy(outf[0:C, sl0], ps_out[0:C, sl0])
            nc.scalar.copy(outf[C : 2 * C, sl1], ps_out[C : 2 * C, sl1])
            nc.sync.dma_start(out=d_out[:, sl0], in_=outf[0:C, sl0])
            nc.sync.dma_start(out=d_out[:, sl1], in_=outf[C : 2 * C, sl1])
            ps_acc_cm.__exit__(None, None, None)
            dram_cm.__exit__(None, None, None)

    nc.compile()
    return nc


def _get_nc():
    if "nc" not in _CACHE:
        _CACHE["nc"] = _build_bass()
    return _CACHE["nc"]


def _host_prep(x, centers, sigmas, weights, biases, bn_gamma, bn_beta, rule_masks):
    xT = np.ascontiguousarray(np.asarray(x, dtype=np.float32).T)  # [D, B]
    wstack = np.ascontiguousarray(
        np.transpose(np.asarray(weights, dtype=np.float32), (1, 0, 2)).reshape(D, R * C)
    )
    common = {
        "xt_full": xT,
        "centers_t": np.ascontiguousarray(np.asarray(centers, np.float32)),
        "sigmas_t": np.ascontiguousarray(np.asarray(sigmas, np.float32)),
        "wstack": wstack,
        "biases2d": np.ascontiguousarray(np.asarray(biases, np.float32)[0]),
        "gamma_c": np.ascontiguousarray(np.asarray(bn_gamma, np.float32).reshape(D, 1)),
        "beta_c": np.ascontiguousarray(np.asarray(bn_beta, np.float32).reshape(D, 1)),
        "masks_c": np.ascontiguousarray(np.asarray(rule_masks, np.float32).reshape(R, 1)),
    }
    in_maps = []
    for m in range(NCORES):
        im = dict(common)
        im["xt_loc"] = np.ascontiguousarray(xT[:, m * BL : (m + 1) * BL])
        in_maps.append(im)
    return in_maps


def run_on_hw(inputs, trace=False, **kw):
    from concourse.bass_utils import run_bass_kernel_spmd

    nc = _get_nc()
    in_maps = _host_prep(**inputs)
    res = run_bass_kernel_spmd(
        nc, in_maps, core_ids=list(range(NCORES)), trace=trace, **kw
    )
    out = np.empty((B, C), dtype=np.float32)
    for m in range(NCORES):
        out[m * BL : (m + 1) * BL, :] = res.results[m]["outT"].T
    return out, res


def kernel(x, centers, sigmas, weights, biases, bn_gamma, bn_beta, rule_masks):
    out, _ = run_on_hw(
        dict(
            x=x, centers=centers, sigmas=sigmas, weights=weights, biases=biases,
            bn_gamma=bn_gamma, bn_beta=bn_beta, rule_masks=rule_masks,
        )
    )
    return out



# revision 11
# speedup vs baseline: 1.1770x; 1.1770x over previous
# Trainium2 Bass kernel for nn_FuzzyNeuralNework (moe_routing).
#
# Math (reference):
#   logits[b,r] = sum_d -(x[b,d]-cen[d,r])^2 / (2 sig[d,r]^2)
#   raw = exp(logits) * mask ;  frs = raw / (sum_r raw + 1e-10)
#   xn = batchnorm(x) (global batch stats, biased var)
#   out[b,c] = sum_r frs[b,r] * (xn @ W[r])[b,c] + sum_r frs[b,r]*bias[r,c]
#
# Layout: batch-on-partitions throughout.
#   logits[128b, 64r] per 128-row chunk via PE with x^T / (x^2)^T chunk
#   stationaries (fp32, exp-sensitive) streaming A=-1/(2s^2), Bc=c/s^2 and a
#   K=1 rank-1 matmul adding k[r]=sum_d -c^2/(2s^2).
#   exp+denominator fused in one ACT op (accum_out reduces over the free r
#   axis); normalization is a per-partition scalar multiply. No partition
#   broadcasts anywhere.
#   cons GEMM: stationary xn^T chunk [128d,128b], moving W permuted c-major
#   [d, (c,r)] -> PSUM [128b, (c,r)] fp32. Gating multiplies PSUM by a
#   zero-stride broadcast view of frs (frs[:,None,:].broadcast_to), writing
#   bf16; rule-reduce = bf16 pair-tree + fp32 tensor_reduce tail on strided
#   views. Evac/gate/tree ops are statically scheduled across ACT/DVE/GPSIMD.
#   BN stats come from a host-cast bf16 replica of x^T (halved DMA); output
#   rows leave in natural [b, c] layout (no host transpose of the result).
#
# Sharding: batch B=8192 split across 8 cores (1024 each); small tensors
# replicated.

import numpy as np

B, D, R, C = 8192, 128, 64, 64
NCORES = 8
BL = B // NCORES
NCH = BL // 128  # 8 chunks of 128 batch rows per core
BN_EPS = 1e-5

_CACHE = {}


def _build_bass(with_bias, with_mask):
    import concourse.bass as bass
    import concourse.tile as tile
    from concourse import bacc, mybir

    f32 = mybir.dt.float32
    bf16 = mybir.dt.bfloat16
    AF = mybir.ActivationFunctionType
    OP = mybir.AluOpType

    nc = bacc.Bacc(
        "TRN2", target_bir_lowering=False, debug=False, num_devices=NCORES
    )

    d_xtl = nc.dram_tensor("xt_loc", [D, BL], f32, kind="ExternalInput").ap()
    d_xtfb = nc.dram_tensor("xt_full_bf", [D, B], bf16, kind="ExternalInput").ap()
    d_wpb = nc.dram_tensor("wperm_bf", [D, C * R], bf16, kind="ExternalInput").ap()
    d_cen = nc.dram_tensor("centers_t", [D, R], f32, kind="ExternalInput").ap()
    d_sig = nc.dram_tensor("sigmas_t", [D, R], f32, kind="ExternalInput").ap()
    d_gam = nc.dram_tensor("gamma_c", [D, 1], f32, kind="ExternalInput").ap()
    d_bet = nc.dram_tensor("beta_c", [D, 1], f32, kind="ExternalInput").ap()
    if with_mask:
        d_mskr = nc.dram_tensor("masks_row", [1, R], f32, kind="ExternalInput").ap()
    if with_bias:
        d_brow = nc.dram_tensor(
            "biases_row_cr", [1, C * R], bf16, kind="ExternalInput"
        ).ap()
    d_out = nc.dram_tensor("out_loc", [BL, C], f32, kind="ExternalOutput").ap()

    # static schedule for the 16 (chunk, half) macro-steps of phase 2:
    # evac engine, gate engine (None = fused into DVE evac), tree plan.
    #  'A' = ACT copy-evac then separate gate; 'V' = DVE fused gated evac.
    #  gate: 'V' dve, 'G' gpsimd.   tree: 'V' dve t1+t2+tail,
    #  'GV' gpsimd t1 then dve t2+tail, 'G' full gpsimd chain.
    sched = []
    for t in range(2 * NCH):
        m = t % 4
        if m == 3:
            sched.append(("V", None, "GV"))
        elif m == 1:
            sched.append(("A", "G", "V"))
        else:  # m in (0, 2)
            sched.append(("A", "V", "G" if m == 0 else "V"))

    with tile.TileContext(nc) as tc:
        with (
            tc.tile_pool(name="consts", bufs=1) as consts,
            tc.tile_pool(name="bigs", bufs=1) as bigs,
            tc.tile_pool(name="gpool", bufs=4) as gpool,
            tc.tile_pool(name="cpool", bufs=3) as cpool,
            tc.tile_pool(name="t1pool", bufs=3) as t1pool,
            tc.tile_pool(name="t2pool", bufs=3) as t2pool,
            tc.tile_pool(name="opool", bufs=2) as opool,
            tc.tile_pool(name="spool", bufs=3) as spool,
        ):
            ps_early_cm = tc.tile_pool(name="ps_early", bufs=2, space="PSUM")
            ps_early = ps_early_cm.__enter__()

            # ---- input DMAs (critical-path first, spread over engines) --
            sb_xtl = bigs.tile([D, BL], f32)
            nc.sync.dma_start(out=sb_xtl, in_=d_xtl)
            sb_cen = consts.tile([D, R], f32)
            sb_sig = consts.tile([D, R], f32)
            nc.scalar.dma_start(out=sb_cen, in_=d_cen)
            nc.scalar.dma_start(out=sb_sig, in_=d_sig)
            sb_gam = consts.tile([D, 1], f32)
            sb_bet = consts.tile([D, 1], f32)
            nc.gpsimd.dma_start(out=sb_gam, in_=d_gam)
            nc.gpsimd.dma_start(out=sb_bet, in_=d_bet)

            sb_wpb = bigs.tile([D, C * R], bf16)
            for h in range(2):
                sl = slice(h * (C * R // 2), (h + 1) * (C * R // 2))
                nc.scalar.dma_start(out=sb_wpb[:, sl], in_=d_wpb[:, sl])
            sb_xtfb = bigs.tile([D, B], bf16)
            dma_engs = [nc.sync, nc.scalar, nc.gpsimd]
            for h in range(4):
                sl = slice(h * (B // 4), (h + 1) * (B // 4))
                dma_engs[h % 3].dma_start(out=sb_xtfb[:, sl], in_=d_xtfb[:, sl])
            if with_mask:
                # replicate mask across partitions via a DRAM broadcast DMA
                sb_mrep = consts.tile([128, R], f32)
                nc.sync.dma_start(
                    out=sb_mrep, in_=d_mskr[0:1, :].to_broadcast((128, R))
                )
            if with_bias:
                # replicate the (c-major) bias row across partitions
                sb_brep = bigs.tile([128, C * R], bf16)
                for h in range(2):
                    sl = slice(h * (C * R // 2), (h + 1) * (C * R // 2))
                    nc.gpsimd.dma_start(
                        out=sb_brep[:, sl],
                        in_=d_brow[0:1, sl].to_broadcast((128, C * R // 2)),
                    )

            # ---- PE warmup (pstate ramp) while DMAs stream in ------------
            warm = consts.tile([D, 128], bf16)
            nc.gpsimd.memset(warm, 0.0)
            warm_ps = ps_early.tile([D, 128], f32)
            for _ in range(24):
                nc.tensor.matmul(warm_ps, warm, warm, start=True, stop=True)

            # ---- Gaussian-membership coefficient prep (tiny DVE ops) ----
            sigsq = consts.tile([D, R], f32)
            nc.vector.tensor_mul(sigsq, sb_sig, sb_sig)
            recs = consts.tile([D, R], f32)
            nc.vector.reciprocal(recs, sigsq)
            sbA = consts.tile([D, R], f32)
            nc.vector.tensor_scalar_mul(sbA, recs, -0.5)
            sbBc = consts.tile([D, R], f32)
            nc.vector.tensor_mul(sbBc, sb_cen, recs)
            csq = consts.tile([D, R], f32)
            nc.vector.tensor_mul(csq, sb_cen, sb_cen)
            cA = consts.tile([D, R], f32)
            nc.vector.tensor_mul(cA, csq, sbA)

            ones_d = consts.tile([D, 1], f32)
            nc.vector.memset(ones_d, 1.0)
            ones_row = consts.tile([1, 128], f32)
            nc.vector.memset(ones_row, 1.0)
            ps_k = ps_early.tile([1, R], f32)
            nc.tensor.matmul(ps_k, ones_d, cA, start=True, stop=True)
            sb_krow = consts.tile([1, R], f32)
            nc.vector.tensor_copy(sb_krow, ps_k)

            # x^2 in fp32 for the logits matmul stationaries
            x2t = bigs.tile([D, BL], f32)
            nc.scalar.activation(x2t, sb_xtl, AF.Square)

            # ---- logits + frs, batch-on-partition, chunk by chunk -------
            frs_sb = bigs.tile([128, NCH * R], bf16)
            raws = []
            for i in range(NCH):
                ci = slice(i * 128, (i + 1) * 128)
                ps_lg = ps_early.tile([128, R], f32)
                nc.tensor.matmul(ps_lg, x2t[:, ci], sbA, start=True, stop=False)
                nc.tensor.matmul(ps_lg, sb_xtl[:, ci], sbBc, start=False, stop=False)
                nc.tensor.matmul(ps_lg, ones_row, sb_krow, start=False, stop=True)
                raw = cpool.tile([128, R], f32)
                den = spool.tile([128, 1], f32)
                if with_mask:
                    nc.scalar.activation(raw, ps_lg, AF.Exp)
                    nc.vector.tensor_mul(raw, raw, sb_mrep)
                    nc.vector.tensor_reduce(
                        out=den, in_=raw, axis=mybir.AxisListType.X, op=OP.add
                    )
                else:
                    nc.scalar.activation(raw, ps_lg, AF.Exp, accum_out=den)
                den_e = spool.tile([128, 1], f32)
                nc.vector.tensor_scalar_add(den_e, den, 1e-10)
                recip = spool.tile([128, 1], f32)
                nc.vector.reciprocal(recip, den_e)
                nc.vector.tensor_scalar_mul(
                    frs_sb[:, i * R : (i + 1) * R], raw, recip
                )

            # ---- BN stats over the full batch (bf16 replica) ------------
            shalf = bigs.tile([D, B // 2], bf16)
            nc.vector.tensor_add(shalf, sb_xtfb[:, : B // 2], sb_xtfb[:, B // 2 :])
            xsum = consts.tile([D, 1], f32)
            nc.vector.tensor_reduce(
                out=xsum, in_=shalf, axis=mybir.AxisListType.X, op=OP.add
            )
            sqscr = bigs.tile([D, B // 2], bf16)
            sq2 = consts.tile([D, 2], f32)
            for h in range(2):
                sl = slice(h * (B // 2), (h + 1) * (B // 2))
                nc.scalar.activation(
                    out=sqscr, in_=sb_xtfb[:, sl], func=AF.Square,
                    accum_out=sq2[:, h : h + 1],
                )
            sqsum = consts.tile([D, 1], f32)
            nc.vector.tensor_reduce(
                out=sqsum, in_=sq2, axis=mybir.AxisListType.X, op=OP.add
            )
            mean = consts.tile([D, 1], f32)
            nc.vector.tensor_scalar_mul(mean, xsum, 1.0 / float(B))
            msq = consts.tile([D, 1], f32)
            nc.vector.tensor_mul(msq, mean, mean)
            var = consts.tile([D, 1], f32)
            nc.vector.tensor_scalar_mul(var, sqsum, 1.0 / float(B))
            nc.vector.tensor_sub(var, var, msq)
            # rstd = exp(-0.5*ln(var+eps)) avoids the low-precision Rsqrt table
            eps_d = consts.tile([D, 1], f32)
            nc.vector.memset(eps_d, float(BN_EPS))
            lnv = consts.tile([D, 1], f32)
            nc.scalar.activation(lnv, var, AF.Ln, bias=eps_d)
            rstd = consts.tile([D, 1], f32)
            nc.scalar.activation(rstd, lnv, AF.Exp, scale=-0.5)
            a_sc = consts.tile([D, 1], f32)
            nc.vector.tensor_mul(a_sc, rstd, sb_gam)
            mu_a = consts.tile([D, 1], f32)
            nc.vector.tensor_mul(mu_a, mean, a_sc)
            c0 = consts.tile([D, 1], f32)
            nc.vector.tensor_sub(c0, sb_bet, mu_a)
            xn_bf = bigs.tile([D, BL], bf16)
            nc.vector.tensor_scalar(
                out=xn_bf, in0=sb_xtl, scalar1=a_sc, scalar2=c0,
                op0=OP.mult, op1=OP.add,
            )

            ps_early_cm.__exit__(None, None, None)

            # ---- gated GEMM + rule reduce, PSUM ping-pong ---------------
            ps_acc_cm = tc.tile_pool(name="ps_acc", bufs=2, space="PSUM")
            ps_acc = ps_acc_cm.__enter__()
            HW2 = C * R // 2  # 2048 columns per half

            with nc.allow_low_precision("bf16 rule-pair tree; fp32 tail"):
                for i in range(NCH):
                    ci = slice(i * 128, (i + 1) * 128)
                    out_sb = opool.tile([128, C], f32)
                    frs_i = frs_sb[:, i * R : (i + 1) * R]
                    for h in range(2):
                        t = i * 2 + h
                        evac_e, gate_e, tree_e = sched[t]
                        psH = ps_acc.tile([128, HW2], f32)
                        for j in range(4):
                            nc.tensor.matmul(
                                psH[:, j * 512 : (j + 1) * 512],
                                xn_bf[:, ci],
                                sb_wpb[:, h * HW2 + j * 512 : h * HW2 + (j + 1) * 512],
                                start=True, stop=True,
                            )
                        ps3 = psH.rearrange("p (c r) -> p c r", r=R)
                        fv = frs_i[:, None, :].broadcast_to([128, 32, R])
                        g = gpool.tile([128, HW2], bf16)
                        g3 = g.rearrange("p (c r) -> p c r", r=R)
                        if with_bias:
                            # cons + bias[r,c] folded in during evacuation
                            cs = cpool.tile([128, HW2], bf16)
                            cs3 = cs.rearrange("p (c r) -> p c r", r=R)
                            br3 = sb_brep[:, h * HW2 : (h + 1) * HW2].rearrange(
                                "p (c r) -> p c r", r=R
                            )
                            nc.vector.tensor_add(cs3, ps3, br3)
                            eng = nc.vector if gate_e == "V" else nc.gpsimd
                            eng.tensor_mul(g3, cs3, fv)
                        elif evac_e == "V":
                            nc.vector.tensor_mul(g3, ps3, fv)
                        else:
                            cs = cpool.tile([128, HW2], bf16)
                            nc.scalar.copy(cs, psH)
                            cs3 = cs.rearrange("p (c r) -> p c r", r=R)
                            eng = nc.vector if gate_e == "V" else nc.gpsimd
                            eng.tensor_mul(g3, cs3, fv)
                        # rule tree: 64 -> 32 -> 16 -> fp32 tail
                        t1 = t1pool.tile([128, 32 * 32], bf16)
                        t1_3 = t1.rearrange("p (c r) -> p c r", r=32)
                        t2 = t2pool.tile([128, 32 * 16], bf16)
                        t2_3 = t2.rearrange("p (c r) -> p c r", r=16)
                        o_h = out_sb[:, h * 32 : (h + 1) * 32]
                        if tree_e == "V":
                            nc.vector.tensor_add(t1_3, g3[:, :, 0:32], g3[:, :, 32:64])
                            nc.vector.tensor_add(
                                t2_3, t1_3[:, :, 0:16], t1_3[:, :, 16:32]
                            )
                            nc.vector.tensor_reduce(
                                out=o_h, in_=t2_3, axis=mybir.AxisListType.X, op=OP.add
                            )
                        elif tree_e == "GV":
                            nc.gpsimd.tensor_add(t1_3, g3[:, :, 0:32], g3[:, :, 32:64])
                            nc.vector.tensor_add(
                                t2_3, t1_3[:, :, 0:16], t1_3[:, :, 16:32]
                            )
                            nc.vector.tensor_reduce(
                                out=o_h, in_=t2_3, axis=mybir.AxisListType.X, op=OP.add
                            )
                        else:  # full gpsimd chain down to 2 rules + fp32 add
                            nc.gpsimd.tensor_add(t1_3, g3[:, :, 0:32], g3[:, :, 32:64])
                            nc.gpsimd.tensor_add(
                                t2_3, t1_3[:, :, 0:16], t1_3[:, :, 16:32]
                            )
                            t3 = t1pool.tile([128, 32 * 8], bf16)
                            t3_3 = t3.rearrange("p (c r) -> p c r", r=8)
                            nc.gpsimd.tensor_add(
                                t3_3, t2_3[:, :, 0:8], t2_3[:, :, 8:16]
                            )
                            nc.vector.tensor_reduce(
                                out=o_h, in_=t3_3, axis=mybir.AxisListType.X, op=OP.add
                            )
                    nc.sync.dma_start(
                        out=d_out[i * 128 : (i + 1) * 128, :], in_=out_sb
                    )
            ps_acc_cm.__exit__(None, None, None)

    nc.compile()
    return nc


def _get_nc(with_bias, with_mask):
    key = ("nc", with_bias, with_mask)
    if key not in _CACHE:
        _CACHE[key] = _build_bass(with_bias, with_mask)
    return _CACHE[key]


def _host_prep(x, centers, sigmas, weights, biases, bn_gamma, bn_beta, rule_masks):
    import ml_dtypes

    bf = ml_dtypes.bfloat16
    xT = np.ascontiguousarray(np.asarray(x, dtype=np.float32).T)  # [D, B]
    # W [R, D, C] -> [D, C, R] (c-major, r-minor) -> [D, C*R] bf16
    wperm = np.ascontiguousarray(
        np.transpose(np.asarray(weights, dtype=np.float32), (1, 2, 0))
        .reshape(D, C * R)
        .astype(bf)
    )
    with_bias = bool(np.any(np.asarray(biases)))
    with_mask = bool(np.any(np.asarray(rule_masks) != 1.0))
    common = {
        "xt_full_bf": np.ascontiguousarray(xT.astype(bf)),
        "wperm_bf": wperm,
        "centers_t": np.ascontiguousarray(np.asarray(centers, np.float32)),
        "sigmas_t": np.ascontiguousarray(np.asarray(sigmas, np.float32)),
        "gamma_c": np.ascontiguousarray(np.asarray(bn_gamma, np.float32).reshape(D, 1)),
        "beta_c": np.ascontiguousarray(np.asarray(bn_beta, np.float32).reshape(D, 1)),
    }
    if with_mask:
        common["masks_row"] = np.ascontiguousarray(
            np.asarray(rule_masks, np.float32).reshape(1, R)
        )
    if with_bias:
        # [R, C] -> c-major row [1, C*R]: value[c*R + r] = biases[0, r, c]
        common["biases_row_cr"] = np.ascontiguousarray(
            np.asarray(biases, np.float32)[0].T.reshape(1, C * R).astype(bf)
        )
    in_maps = []
    for m in range(NCORES):
        im = dict(common)
        im["xt_loc"] = np.ascontiguousarray(xT[:, m * BL : (m + 1) * BL])
        in_maps.append(im)
    return in_maps, with_bias, with_mask


def run_on_hw(inputs, trace=False, **kw):
    from concourse.bass_utils import run_bass_kernel_spmd

    in_maps, with_bias, with_mask = _host_prep(**inputs)
    nc = _get_nc(with_bias, with_mask)
    res = run_bass_kernel_spmd(
        nc, in_maps, core_ids=list(range(NCORES)), trace=trace, **kw
    )
    out = np.empty((B, C), dtype=np.float32)
    for m in range(NCORES):
        out[m * BL : (m + 1) * BL, :] = res.results[m]["out_loc"]
    return out, res


def kernel(x, centers, sigmas, weights, biases, bn_gamma, bn_beta, rule_masks):
    out, _ = run_on_hw(
        dict(
            x=x, centers=centers, sigmas=sigmas, weights=weights, biases=biases,
            bn_gamma=bn_gamma, bn_beta=bn_beta, rule_masks=rule_masks,
        )
    )
    return out
